# revision 76
# baseline (speedup 1.0000x reference)
"""Multi-head causal attention (B=2, T=2048, D=1024, H=16) on 8 TRN2
NeuronCores: data parallel over batch x tensor parallel over head groups
(4 heads per core). Each core computes its group's Q/K/V projections,
causal attention, and a partial output projection; the host sums the 4
partials per batch element.

v2: bf16 datapath (host-converted inputs), row-tiled S-matmul head pairs
(two 64-contraction matmuls run concurrently in the PE array), V^T
produced directly by swapping matmul operands (no PE transposes), fast
approximate reciprocal for the softmax denominators, paired broadcast
and divide.

Self-contained: builds the Bass/Tile kernel, runs it via
run_bass_kernel_spmd on cores 0-7, gathers on host.
"""
import numpy as np
import ml_dtypes

import concourse.bass as bass
import concourse.mybir as mybir
import concourse.tile as tile
from concourse.bass_utils import run_bass_kernel_spmd

P = 128
B, T, D = 2, 2048, 1024
H_LOCAL = 4          # heads per core
HD = 64              # head dim
F = H_LOCAL * HD     # 256 features per group
KO = D // P          # 8 contraction subtiles
NT = 512             # matmul moving width / PSUM bank
QJ = T // NT         # 4 q column tiles
KT = T // P          # 16 k row tiles
N_CORES = 8
LAG = 3              # S-matmul lookahead over P@V accumulation

f32 = mybir.dt.float32
f32r = mybir.dt.float32r
bf16 = mybir.dt.bfloat16

_uid = [0]


def _legalize_single_wait(nc):
    # This walrus build accepts only ONE sem wait per instruction; hoist
    # extra waits onto single-wait NoOps placed just before the instruction.
    for fn in nc.m.functions:
        for bb in fn.blocks:
            new_list = []
            changed = False
            for inst in bb.instructions:
                si = inst.sync_info
                if si is not None and len(si.on_wait) > 1:
                    waits = list(si.on_wait)
                    for w in waits[:-1]:
                        _uid[0] += 1
                        new_list.append(mybir.InstNoOp(
                            name=f"I-waitsplit-{_uid[0]}",
                            engine=inst.engine,
                            sync_info=mybir.SyncInfo(on_wait=[w], on_update=[]),
                        ))
                    inst.sync_info = mybir.SyncInfo(
                        on_wait=[waits[-1]], on_update=list(si.on_update))
                    changed = True
                new_list.append(inst)
            if changed:
                bb.instructions.clear()
                bb.instructions.extend(new_list)


def build_nc():
    nc = bass.Bass(trn_type="TRN2", target_bir_lowering=False, debug=False,
                   num_devices=N_CORES)
    xT = nc.dram_tensor("xT", [D, T], bf16, kind="ExternalInput").ap()
    WqT = nc.dram_tensor("WqT", [D, F], bf16, kind="ExternalInput").ap()
    WkT = nc.dram_tensor("WkT", [D, F], bf16, kind="ExternalInput").ap()
    WvT = nc.dram_tensor("WvT", [D, F], bf16, kind="ExternalInput").ap()
    WoT = nc.dram_tensor("WoT", [F, D], bf16, kind="ExternalInput").ap()
    TRI = nc.dram_tensor("TRI", [P, P], bf16, kind="ExternalInput").ap()
    Z = nc.dram_tensor("Z", [T, D], bf16, kind="ExternalOutput").ap()

    xTr = xT.rearrange("(ko p) t -> p ko t", p=P)
    w_r = {
        "q": WqT.rearrange("(ko p) f -> p ko f", p=P),
        "k": WkT.rearrange("(ko p) f -> p ko f", p=P),
        "v": WvT.rearrange("(ko p) f -> p ko f", p=P),
    }

    with tile.TileContext(nc) as tc:
        with (
            tc.tile_pool(name="cw", bufs=1) as cw,
            tc.tile_pool(name="sb1", bufs=1) as sb1,
            tc.tile_pool(name="tp", bufs=4) as tp,
            tc.tile_pool(name="psS", bufs=4, space="PSUM") as psS,
            tc.tile_pool(name="psO", bufs=2, space="PSUM") as psO,
            tc.tile_pool(name="psM", bufs=2, space="PSUM") as psM,
        ):
            # ---- persistent constants / staging ----
            w_sb = {}
            for name in ("q", "k", "v"):
                w_sb[name] = sb1.tile([P, KO, F], bf16, tag=f"w{name}",
                                      name=f"w{name}")
            xt = sb1.tile([P, KO, T], bf16, tag="xt", name="xt")
            # batched whole-tensor transfers (large DMAs spray across the
            # DMA engines; per-ko issues cost ~0.6us each on the queue);
            # weights dispatch on Sync, x on ACT (idle at startup) so the
            # descriptors go out in parallel
            nc.sync.dma_start(w_sb["q"][:], w_r["q"][:])
            nc.scalar.dma_start(xt[:, :, 0:NT], xTr[:, :, 0:NT])
            nc.sync.dma_start(w_sb["k"][:], w_r["k"][:])
            nc.scalar.dma_start(xt[:, :, NT:2 * NT], xTr[:, :, NT:2 * NT])
            nc.sync.dma_start(w_sb["v"][:], w_r["v"][:])
            for qj in range(2, QJ):
                nc.sync.dma_start(xt[:, :, qj * NT:(qj + 1) * NT],
                                  xTr[:, :, qj * NT:(qj + 1) * NT])

            wo = cw.tile([P, F // P, D], bf16, tag="wo", name="wo")
            nc.gpsimd.dma_start(wo[:], WoT.rearrange("(fo p) d -> p fo d", p=P))
            # causal mask replicated for the two heads of a pair
            tri2 = cw.tile([P, 2, P], bf16, tag="tri2", name="tri2")
            nc.gpsimd.dma_start(tri2[:, 0], TRI)
            nc.gpsimd.dma_start(tri2[:, 1], TRI)
            ones33 = cw.tile([1, HD], mybir.dt.float16, tag="ones33",
                             name="ones33")
            nc.gpsimd.memset(ones33[:], 1.0)

            # Q/K^T for head pair p: rows 0:64 = head 2p, rows 64:128 = head
            # 2p+1 (the projection psum layout, verbatim).
            qt = cw.tile([P, 2, T], bf16, tag="qt", name="qt")
            kt2 = cw.tile([P, 2, T], bf16, tag="kt2", name="kt2")

            # V^T with a ones column per head: [k-token, kt, head, 0:64]=V^T,
            # [..., 64]=1 (gives softmax denominators for free in P@V)
            vaug = cw.tile([P, KT, H_LOCAL, HD + 1], bf16, tag="vaug",
                           name="vaug")
            nc.gpsimd.memset(vaug[:, :, :, HD:HD + 1], 1.0)

            ot = cw.tile([P, F // P, T], bf16, tag="ot", name="ot")

            def phase1_chunks(qj):
                # emission chunks (each ~8 PE matmuls) to splice into the
                # attention stream so the PE never drains
                sl = slice(qj * NT, (qj + 1) * NT)
                chunks = []

                def proj(name, fs):
                    def emit():
                        ps = psM.tile([P, NT], f32, tag="m",
                                      name=f"ps_{name}{fs}_{qj}")
                        for ko in range(KO):
                            nc.tensor.matmul(
                                ps[:], w_sb[name][:, ko, fs * P:(fs + 1) * P],
                                xt[:, ko, sl],
                                start=(ko == 0), stop=(ko == KO - 1))
                        dst = qt if name == "q" else kt2
                        nc.vector.tensor_copy(dst[:, fs, sl], ps[:])
                    return emit

                def vproj(kt):
                    # V^T directly: stationary = x tile, moving = Wv.
                    # out[tok, f] = sum_d x[kt*128+tok, d] * Wv[f, d]
                    def emit():
                        pv = psM.tile([P, F], f32, tag="m", name=f"pv{kt}")
                        for ko in range(KO):
                            nc.tensor.matmul(
                                pv[:], xt[:, ko, kt * P:(kt + 1) * P],
                                w_sb["v"][:, ko, :],
                                start=(ko == 0), stop=(ko == KO - 1))
                        nc.vector.tensor_copy(
                            vaug[:, kt, :, 0:HD],
                            pv.rearrange("p (h d) -> p h d", h=H_LOCAL))
                    return emit

                for name in ("q", "k"):
                    for fs in range(F // P):
                        chunks.append(proj(name, fs))
                for kt in range(4 * qj, 4 * qj + 4):
                    chunks.append(vproj(kt))
                return chunks

            def phase23_pair(p, qj, pending, splice):
                # two heads (2p, 2p+1) processed together: their S matmuls
                # are 64-contraction row-tiles (partitions 0:64 / 64:128)
                # that run concurrently in the PE array.
                po = [psO.tile([HD + 1, NT], f32, tag="o",
                               name=f"po{p}_{qj}_{e}") for e in (0, 1)]
                n_ki = 4 * qj + 4
                pts = {}

                def s_step(ki):
                    col0 = 0 if ki < 4 * qj else (ki - 4 * qj) * P
                    N = NT - col0
                    kb = slice(ki * P, (ki + 1) * P)
                    qs = slice(qj * NT + col0, (qj + 1) * NT)
                    # the pair's S matmuls write one 2-bank PSUM tile: both
                    # banks recycle together, so the two row-tiled matmuls
                    # become ready together and run concurrently; ONE merged
                    # exp and ONE merged mask-mul cover both heads
                    ps = psS.tile([P, 2, NT], f32, tag="s", bufs=2,
                                  name=f"pss{p}_{qj}_{ki}")
                    for e in (0, 1):
                        rows = slice(HD * e, HD * e + HD)
                        nc.tensor.matmul(
                            ps[:, e, 0:N], kt2[rows, p, kb], qt[rows, p, qs],
                            start=True, stop=True)
                    pt = tp.tile([P, 2, NT], bf16, tag="pt", bufs=4,
                                 name=f"pt{p}_{qj}_{ki}")
                    nc.scalar.activation(pt[:, :, 0:N], ps[:, :, 0:N],
                                         mybir.ActivationFunctionType.Exp,
                                         scale=0.125)
                    if ki >= 4 * qj:
                        nc.vector.tensor_mul(pt[:, :, 0:P], pt[:, :, 0:P],
                                             tri2[:])
                    pts[ki] = (pt, col0, N)

                def o_step(ki):
                    pt, col0, N = pts.pop(ki)
                    for e in (0, 1):
                        nc.tensor.matmul(
                            po[e][:, col0:NT], vaug[:, ki, 2 * p + e, :],
                            pt[:, e, 0:N],
                            start=(ki == 0), stop=(ki == n_ki - 1))

                # splice points: external chunks between ki steps. The
                # first LAG iterations have no o_steps (PE would sit ~50%
                # idle there and HAM can re-throttle), so they get a double
                # share of the chunks.
                nst = n_ki + LAG
                wts = [2 if ki < LAG else 1 for ki in range(nst)]
                tot = sum(wts)
                cum = [0]
                for w in wts:
                    cum.append(cum[-1] + w)
                nsp = len(splice)

                for ki in range(nst):
                    if ki < n_ki:
                        s_step(ki)
                    if ki == LAG and pending is not None:
                        # previous pair's division, emitted here so its PE
                        # broadcast never heads the PE stream while waiting
                        # on the ACT reciprocal chain
                        pending()
                        pending = None
                    if ki >= LAG:
                        o_step(ki - LAG)
                    for c in splice[(nsp * cum[ki]) // tot:
                                    (nsp * cum[ki + 1]) // tot]:
                        c()
                if pending is not None:
                    pending()

                def division():
                    # numerators of both heads stacked [128, NT]; raw
                    # denominator rows broadcast across partitions by two
                    # col-tiled PE matmuls; one paired DVE divide into OT.
                    sp = tp.tile([P, NT], f32, tag="so", bufs=2,
                                 name=f"sp{p}_{qj}")
                    rrs = []
                    for e in (0, 1):
                        nc.vector.tensor_copy(sp[HD * e:HD * e + HD, :],
                                              po[e][0:HD, :])
                        # 1/d = exp(-ln d) on ACT (DVE reciprocal is serial
                        # per-lane; custom DVE ops unsupported in this build)
                        ll = tp.tile([1, NT], f32, tag=f"ll{e}", bufs=2,
                                     name=f"ll{p}_{qj}_{e}")
                        nc.scalar.activation(ll[:], po[e][HD:HD + 1, :],
                                             mybir.ActivationFunctionType.Ln)
                        rr = tp.tile([1, NT], mybir.dt.float16, tag=f"rr{e}",
                                     bufs=2, name=f"rr{p}_{qj}_{e}")
                        nc.scalar.activation(rr[:], ll[:],
                                             mybir.ActivationFunctionType.Exp,
                                             scale=-1.0)
                        rrs.append(rr)
                    pb = psM.tile([P, NT], f32, tag="m", name=f"pb{p}_{qj}")
                    for e in (0, 1):
                        nc.tensor.matmul(pb[HD * e:HD * e + HD, :],
                                         ones33[0:1, :], rrs[e][:],
                                         start=True, stop=True)
                    nc.vector.tensor_mul(ot[:, p, qj * NT:(qj + 1) * NT],
                                         sp[:], pb[:])
                return division

            def phase4(qt_i):
                for dt in range(D // NT):
                    pz = psM.tile([P, NT], f32, tag="m", name=f"pz{qt_i}_{dt}")
                    for fs in range(F // P):
                        nc.tensor.matmul(
                            pz[:], ot[:, fs, qt_i * P:(qt_i + 1) * P],
                            wo[:, fs, dt * NT:(dt + 1) * NT],
                            start=(fs == 0), stop=(fs == F // P - 1))
                    zs = tp.tile([P, NT], bf16, tag="z", bufs=2,
                                 name=f"zs{qt_i}_{dt}")
                    nc.vector.tensor_copy(zs[:], pz[:])
                    nc.sync.dma_start(
                        Z[qt_i * P:(qt_i + 1) * P, dt * NT:(dt + 1) * NT],
                        zs[:])

            # chunk order within a splice: fs0 projections first (feed the
            # NEXT qj's pair0), early V tiles before the o_steps that read
            # them, fs1 projections in the pair1 half.
            def ordered(ch):
                return [ch[0], ch[2], ch[4], ch[5], ch[1], ch[3], ch[6], ch[7]]

            pending = None
            ch0 = phase1_chunks(0)
            for c in (ch0[0], ch0[2], ch0[4]):   # q fs0, k fs0, V kt0
                c()
            rest0 = [ch0[5], ch0[6], ch0[7], ch0[1], ch0[3]]
            p4_backlog = []
            for qj in range(QJ):
                splice = ordered(phase1_chunks(qj + 1)) if qj + 1 < QJ else []
                if qj == 2:
                    splice += p4_backlog[:4]       # phase4 of qj 0
                    p4_backlog = p4_backlog[4:]
                elif qj == 3:
                    splice += p4_backlog[:4]       # phase4 of qj 1; qj 2's
                    p4_backlog = p4_backlog[4:]    # stays for the tail,
                                                   # ahead of the final
                                                   # division
                if qj == 0:
                    # minimal warmup happened above; attention starts ASAP
                    pending = phase23_pair(0, 0, pending, rest0)
                    pending = phase23_pair(1, 0, pending, splice)
                else:
                    for p in range(2):
                        k0 = (len(splice) * p) // 2
                        k1 = (len(splice) * (p + 1)) // 2
                        pending = phase23_pair(p, qj, pending, splice[k0:k1])
                p4_backlog += [(lambda qt_i=qt_i: phase4(qt_i))
                               for qt_i in range(4 * qj, 4 * qj + 4)]
            # qt 8-11 don't depend on the final division — emit them BEFORE
            # it so they aren't stuck in the PE queue behind the division's
            # broadcast matmuls (which wait on the ACT reciprocal chain)
            for c in p4_backlog[:4]:
                c()
            if pending is not None:
                pending()
            for c in p4_backlog[4:]:
                c()

    _legalize_single_wait(nc)
    return nc


_TRI = None


def _make_in_maps(x, Wq, Wk, Wv, Wo):
    global _TRI
    bf = ml_dtypes.bfloat16
    if _TRI is None:
        # allowed[k_row, q_col] = q >= k  (upper-triangular incl. diagonal)
        _TRI = (np.arange(P)[None, :] >= np.arange(P)[:, None]).astype(bf)
    in_maps = []
    for c in range(N_CORES):
        b, g = divmod(c, 4)
        sl = slice(g * F, (g + 1) * F)
        in_maps.append({
            "xT": np.ascontiguousarray(np.asarray(x)[b].T).astype(bf),
            "WqT": np.ascontiguousarray(np.asarray(Wq)[sl, :].T).astype(bf),
            "WkT": np.ascontiguousarray(np.asarray(Wk)[sl, :].T).astype(bf),
            "WvT": np.ascontiguousarray(np.asarray(Wv)[sl, :].T).astype(bf),
            "WoT": np.ascontiguousarray(np.asarray(Wo)[:, sl].T).astype(bf),
            "TRI": _TRI,
        })
    return in_maps


def run(x, Wq, Wk, Wv, Wo, trace=False, trace_cores=None):
    nc = build_nc()
    in_maps = _make_in_maps(x, Wq, Wk, Wv, Wo)
    res = run_bass_kernel_spmd(nc, in_maps, list(range(N_CORES)), trace=trace,
                               trace_cores=trace_cores)
    out = np.zeros((B, T, D), np.float32)
    for c in range(N_CORES):
        out[c // 4] += np.asarray(res.results[c]["Z"]).astype(np.float32)
    return out, res


def kernel(x, Wq, Wk, Wv, Wo):
    try:
        out, _ = run(x, Wq, Wk, Wv, Wo)
    except Exception:
        # one retry for transient device errors (e.g. a wedged core from a
        # prior run)
        out, _ = run(x, Wq, Wk, Wv, Wo)
    return out


# revision 79
# speedup vs baseline: 1.0266x; 1.0266x over previous
"""Multi-head causal attention (B=2, T=2048, D=1024, H=16) on 8 TRN2
NeuronCores: data parallel over batch x tensor parallel over head groups
(4 heads per core). Each core computes its group's Q/K/V projections,
causal attention, and a partial output projection; the host sums the 4
partials per batch element.

v2: bf16 datapath (host-converted inputs), row-tiled S-matmul head pairs
(two 64-contraction matmuls run concurrently in the PE array), V^T
produced directly by swapping matmul operands (no PE transposes), fast
approximate reciprocal for the softmax denominators, paired broadcast
and divide.

Self-contained: builds the Bass/Tile kernel, runs it via
run_bass_kernel_spmd on cores 0-7, gathers on host.
"""
import numpy as np
import ml_dtypes

import concourse.bass as bass
import concourse.mybir as mybir
import concourse.tile as tile
from concourse.bass_utils import run_bass_kernel_spmd

P = 128
B, T, D = 2, 2048, 1024
H_LOCAL = 4          # heads per core
HD = 64              # head dim
F = H_LOCAL * HD     # 256 features per group
KO = D // P          # 8 contraction subtiles
NT = 512             # matmul moving width / PSUM bank
QJ = T // NT         # 4 q column tiles
KT = T // P          # 16 k row tiles
N_CORES = 8
LAG = 3              # S-matmul lookahead over P@V accumulation

f32 = mybir.dt.float32
f32r = mybir.dt.float32r
bf16 = mybir.dt.bfloat16

_uid = [0]


def _legalize_single_wait(nc):
    # This walrus build accepts only ONE sem wait per instruction; hoist
    # extra waits onto single-wait NoOps placed just before the instruction.
    for fn in nc.m.functions:
        for bb in fn.blocks:
            new_list = []
            changed = False
            for inst in bb.instructions:
                si = inst.sync_info
                if si is not None and len(si.on_wait) > 1:
                    waits = list(si.on_wait)
                    for w in waits[:-1]:
                        _uid[0] += 1
                        new_list.append(mybir.InstNoOp(
                            name=f"I-waitsplit-{_uid[0]}",
                            engine=inst.engine,
                            sync_info=mybir.SyncInfo(on_wait=[w], on_update=[]),
                        ))
                    inst.sync_info = mybir.SyncInfo(
                        on_wait=[waits[-1]], on_update=list(si.on_update))
                    changed = True
                new_list.append(inst)
            if changed:
                bb.instructions.clear()
                bb.instructions.extend(new_list)


def build_nc():
    nc = bass.Bass(trn_type="TRN2", target_bir_lowering=False, debug=False,
                   num_devices=N_CORES)
    xT = nc.dram_tensor("xT", [D, T], bf16, kind="ExternalInput").ap()
    WqT = nc.dram_tensor("WqT", [D, F], bf16, kind="ExternalInput").ap()
    WkT = nc.dram_tensor("WkT", [D, F], bf16, kind="ExternalInput").ap()
    WvT = nc.dram_tensor("WvT", [D, F], bf16, kind="ExternalInput").ap()
    WoT = nc.dram_tensor("WoT", [F, D], bf16, kind="ExternalInput").ap()
    TRI = nc.dram_tensor("TRI", [P, P], bf16, kind="ExternalInput").ap()
    Z = nc.dram_tensor("Z", [T, D], bf16, kind="ExternalOutput").ap()

    xTr = xT.rearrange("(ko p) t -> p ko t", p=P)
    w_r = {
        "q": WqT.rearrange("(ko p) f -> p ko f", p=P),
        "k": WkT.rearrange("(ko p) f -> p ko f", p=P),
        "v": WvT.rearrange("(ko p) f -> p ko f", p=P),
    }

    with tile.TileContext(nc) as tc:
        with (
            tc.tile_pool(name="cw", bufs=1) as cw,
            tc.tile_pool(name="sb1", bufs=1) as sb1,
            tc.tile_pool(name="tp", bufs=4) as tp,
            tc.tile_pool(name="psS", bufs=4, space="PSUM") as psS,
            tc.tile_pool(name="psO", bufs=2, space="PSUM") as psO,
            tc.tile_pool(name="psM", bufs=2, space="PSUM") as psM,
        ):
            # ---- persistent constants / staging ----
            w_sb = {}
            for name in ("q", "k", "v"):
                w_sb[name] = sb1.tile([P, KO, F], bf16, tag=f"w{name}",
                                      name=f"w{name}")
            xt = sb1.tile([P, KO, T], bf16, tag="xt", name="xt")
            # batched whole-tensor transfers (large DMAs spray across the
            # DMA engines; per-ko issues cost ~0.6us each on the queue);
            # weights dispatch on Sync, x on ACT (idle at startup) so the
            # descriptors go out in parallel
            nc.sync.dma_start(w_sb["q"][:], w_r["q"][:])
            # the first projection chunk waits on ALL of xt@qj0 — split it
            # across both DMA-capable engines so the halves transfer in
            # parallel
            nc.scalar.dma_start(xt[:, 0:4, 0:NT], xTr[:, 0:4, 0:NT])
            nc.sync.dma_start(xt[:, 4:8, 0:NT], xTr[:, 4:8, 0:NT])
            nc.sync.dma_start(w_sb["k"][:], w_r["k"][:])
            nc.scalar.dma_start(xt[:, :, NT:2 * NT], xTr[:, :, NT:2 * NT])
            nc.sync.dma_start(w_sb["v"][:], w_r["v"][:])
            for qj in range(2, QJ):
                nc.sync.dma_start(xt[:, :, qj * NT:(qj + 1) * NT],
                                  xTr[:, :, qj * NT:(qj + 1) * NT])

            wo = cw.tile([P, F // P, D], bf16, tag="wo", name="wo")
            nc.gpsimd.dma_start(wo[:], WoT.rearrange("(fo p) d -> p fo d", p=P))
            # causal mask replicated for the two heads of a pair
            tri2 = cw.tile([P, 2, P], bf16, tag="tri2", name="tri2")
            nc.gpsimd.dma_start(tri2[:, 0], TRI)
            nc.gpsimd.dma_start(tri2[:, 1], TRI)
            ones33 = cw.tile([1, HD], mybir.dt.float16, tag="ones33",
                             name="ones33")
            nc.gpsimd.memset(ones33[:], 1.0)

            # Q/K^T for head pair p: rows 0:64 = head 2p, rows 64:128 = head
            # 2p+1 (the projection psum layout, verbatim).
            qt = cw.tile([P, 2, T], bf16, tag="qt", name="qt")
            kt2 = cw.tile([P, 2, T], bf16, tag="kt2", name="kt2")

            # V^T with a ones column per head: [k-token, kt, head, 0:64]=V^T,
            # [..., 64]=1 (gives softmax denominators for free in P@V)
            vaug = cw.tile([P, KT, H_LOCAL, HD + 1], bf16, tag="vaug",
                           name="vaug")
            nc.gpsimd.memset(vaug[:, :, :, HD:HD + 1], 1.0)

            ot = cw.tile([P, F // P, T], bf16, tag="ot", name="ot")

            def phase1_chunks(qj):
                # emission chunks (each ~8 PE matmuls) to splice into the
                # attention stream so the PE never drains
                sl = slice(qj * NT, (qj + 1) * NT)
                chunks = []

                def proj(name, fs):
                    def emit():
                        ps = psM.tile([P, NT], f32, tag="m",
                                      name=f"ps_{name}{fs}_{qj}")
                        for ko in range(KO):
                            nc.tensor.matmul(
                                ps[:], w_sb[name][:, ko, fs * P:(fs + 1) * P],
                                xt[:, ko, sl],
                                start=(ko == 0), stop=(ko == KO - 1))
                        dst = qt if name == "q" else kt2
                        nc.vector.tensor_copy(dst[:, fs, sl], ps[:])
                    return emit

                def vproj(kt):
                    # V^T directly: stationary = x tile, moving = Wv.
                    # out[tok, f] = sum_d x[kt*128+tok, d] * Wv[f, d]
                    def emit():
                        pv = psM.tile([P, F], f32, tag="m", name=f"pv{kt}")
                        for ko in range(KO):
                            nc.tensor.matmul(
                                pv[:], xt[:, ko, kt * P:(kt + 1) * P],
                                w_sb["v"][:, ko, :],
                                start=(ko == 0), stop=(ko == KO - 1))
                        nc.vector.tensor_copy(
                            vaug[:, kt, :, 0:HD],
                            pv.rearrange("p (h d) -> p h d", h=H_LOCAL))
                    return emit

                for name in ("q", "k"):
                    for fs in range(F // P):
                        chunks.append(proj(name, fs))
                for kt in range(4 * qj, 4 * qj + 4):
                    chunks.append(vproj(kt))
                return chunks

            def phase23_pair(p, qj, pending, splice):
                # two heads (2p, 2p+1) processed together: their S matmuls
                # are 64-contraction row-tiles (partitions 0:64 / 64:128)
                # that run concurrently in the PE array.
                po = [psO.tile([HD + 1, NT], f32, tag="o",
                               name=f"po{p}_{qj}_{e}") for e in (0, 1)]
                n_ki = 4 * qj + 4
                pts = {}

                def s_step(ki):
                    col0 = 0 if ki < 4 * qj else (ki - 4 * qj) * P
                    N = NT - col0
                    kb = slice(ki * P, (ki + 1) * P)
                    qs = slice(qj * NT + col0, (qj + 1) * NT)
                    # the pair's S matmuls write one 2-bank PSUM tile: both
                    # banks recycle together, so the two row-tiled matmuls
                    # become ready together and run concurrently; ONE merged
                    # exp and ONE merged mask-mul cover both heads
                    ps = psS.tile([P, 2, NT], f32, tag="s", bufs=2,
                                  name=f"pss{p}_{qj}_{ki}")
                    for e in (0, 1):
                        rows = slice(HD * e, HD * e + HD)
                        nc.tensor.matmul(
                            ps[:, e, 0:N], kt2[rows, p, kb], qt[rows, p, qs],
                            start=True, stop=True)
                    pt = tp.tile([P, 2, NT], bf16, tag="pt", bufs=4,
                                 name=f"pt{p}_{qj}_{ki}")
                    nc.scalar.activation(pt[:, :, 0:N], ps[:, :, 0:N],
                                         mybir.ActivationFunctionType.Exp,
                                         scale=0.125)
                    if ki >= 4 * qj:
                        nc.vector.tensor_mul(pt[:, :, 0:P], pt[:, :, 0:P],
                                             tri2[:])
                    pts[ki] = (pt, col0, N)

                def o_step(ki):
                    pt, col0, N = pts.pop(ki)
                    for e in (0, 1):
                        nc.tensor.matmul(
                            po[e][:, col0:NT], vaug[:, ki, 2 * p + e, :],
                            pt[:, e, 0:N],
                            start=(ki == 0), stop=(ki == n_ki - 1))

                # splice points: external chunks between ki steps. The
                # first LAG iterations have no o_steps (PE would sit ~50%
                # idle there and HAM can re-throttle), so they get a double
                # share of the chunks.
                nst = n_ki + LAG
                wts = [2 if ki < LAG else 1 for ki in range(nst)]
                tot = sum(wts)
                cum = [0]
                for w in wts:
                    cum.append(cum[-1] + w)
                nsp = len(splice)

                for ki in range(nst):
                    if ki < n_ki:
                        s_step(ki)
                    if ki == LAG and pending is not None:
                        # previous pair's division, emitted here so its PE
                        # broadcast never heads the PE stream while waiting
                        # on the ACT reciprocal chain
                        pending()
                        pending = None
                    if ki >= LAG:
                        o_step(ki - LAG)
                    for c in splice[(nsp * cum[ki]) // tot:
                                    (nsp * cum[ki + 1]) // tot]:
                        c()
                if pending is not None:
                    pending()

                def division():
                    # numerators of both heads stacked [128, NT]; raw
                    # denominator rows broadcast across partitions by two
                    # col-tiled PE matmuls; one paired DVE divide into OT.
                    sp = tp.tile([P, NT], f32, tag="so", bufs=2,
                                 name=f"sp{p}_{qj}")
                    rrs = []
                    for e in (0, 1):
                        nc.vector.tensor_copy(sp[HD * e:HD * e + HD, :],
                                              po[e][0:HD, :])
                        # 1/d = exp(-ln d) on ACT (DVE reciprocal is serial
                        # per-lane; custom DVE ops unsupported in this build)
                        ll = tp.tile([1, NT], f32, tag=f"ll{e}", bufs=2,
                                     name=f"ll{p}_{qj}_{e}")
                        nc.scalar.activation(ll[:], po[e][HD:HD + 1, :],
                                             mybir.ActivationFunctionType.Ln)
                        rr = tp.tile([1, NT], mybir.dt.float16, tag=f"rr{e}",
                                     bufs=2, name=f"rr{p}_{qj}_{e}")
                        nc.scalar.activation(rr[:], ll[:],
                                             mybir.ActivationFunctionType.Exp,
                                             scale=-1.0)
                        rrs.append(rr)
                    pb = psM.tile([P, NT], f32, tag="m", name=f"pb{p}_{qj}")
                    for e in (0, 1):
                        nc.tensor.matmul(pb[HD * e:HD * e + HD, :],
                                         ones33[0:1, :], rrs[e][:],
                                         start=True, stop=True)
                    nc.vector.tensor_mul(ot[:, p, qj * NT:(qj + 1) * NT],
                                         sp[:], pb[:])
                return division

            def phase4(qt_i):
                for dt in range(D // NT):
                    pz = psM.tile([P, NT], f32, tag="m", name=f"pz{qt_i}_{dt}")
                    for fs in range(F // P):
                        nc.tensor.matmul(
                            pz[:], ot[:, fs, qt_i * P:(qt_i + 1) * P],
                            wo[:, fs, dt * NT:(dt + 1) * NT],
                            start=(fs == 0), stop=(fs == F // P - 1))
                    zs = tp.tile([P, NT], bf16, tag="z", bufs=2,
                                 name=f"zs{qt_i}_{dt}")
                    nc.vector.tensor_copy(zs[:], pz[:])
                    nc.sync.dma_start(
                        Z[qt_i * P:(qt_i + 1) * P, dt * NT:(dt + 1) * NT],
                        zs[:])

            # chunk order within a splice: fs0 projections first (feed the
            # NEXT qj's pair0), early V tiles before the o_steps that read
            # them, fs1 projections in the pair1 half.
            def ordered(ch):
                return [ch[0], ch[2], ch[4], ch[5], ch[1], ch[3], ch[6], ch[7]]

            pending = None
            ch0 = phase1_chunks(0)
            for c in (ch0[0], ch0[2], ch0[4]):   # q fs0, k fs0, V kt0
                c()
            rest0 = [ch0[5], ch0[6], ch0[7], ch0[1], ch0[3]]
            p4_backlog = []
            for qj in range(QJ):
                splice = ordered(phase1_chunks(qj + 1)) if qj + 1 < QJ else []
                if qj == 2:
                    splice += p4_backlog[:4]       # phase4 of qj 0
                    p4_backlog = p4_backlog[4:]
                elif qj == 3:
                    splice += p4_backlog[:8]       # phase4 of qj 1 and 2 —
                    p4_backlog = p4_backlog[8:]    # keep the tail short and
                                                   # dense so HAM stays warm
                if qj == 0:
                    # minimal warmup happened above; attention starts ASAP
                    pending = phase23_pair(0, 0, pending, rest0)
                    pending = phase23_pair(1, 0, pending, splice)
                else:
                    for p in range(2):
                        k0 = (len(splice) * p) // 2
                        k1 = (len(splice) * (p + 1)) // 2
                        pending = phase23_pair(p, qj, pending, splice[k0:k1])
                p4_backlog += [(lambda qt_i=qt_i: phase4(qt_i))
                               for qt_i in range(4 * qj, 4 * qj + 4)]
            if pending is not None:
                pending()
            for c in p4_backlog:
                c()

    _legalize_single_wait(nc)
    return nc


_TRI = None


def _make_in_maps(x, Wq, Wk, Wv, Wo):
    global _TRI
    bf = ml_dtypes.bfloat16
    if _TRI is None:
        # allowed[k_row, q_col] = q >= k  (upper-triangular incl. diagonal)
        _TRI = (np.arange(P)[None, :] >= np.arange(P)[:, None]).astype(bf)
    in_maps = []
    for c in range(N_CORES):
        b, g = divmod(c, 4)
        sl = slice(g * F, (g + 1) * F)
        in_maps.append({
            "xT": np.ascontiguousarray(np.asarray(x)[b].T).astype(bf),
            "WqT": np.ascontiguousarray(np.asarray(Wq)[sl, :].T).astype(bf),
            "WkT": np.ascontiguousarray(np.asarray(Wk)[sl, :].T).astype(bf),
            "WvT": np.ascontiguousarray(np.asarray(Wv)[sl, :].T).astype(bf),
            "WoT": np.ascontiguousarray(np.asarray(Wo)[:, sl].T).astype(bf),
            "TRI": _TRI,
        })
    return in_maps


def run(x, Wq, Wk, Wv, Wo, trace=False, trace_cores=None):
    nc = build_nc()
    in_maps = _make_in_maps(x, Wq, Wk, Wv, Wo)
    res = run_bass_kernel_spmd(nc, in_maps, list(range(N_CORES)), trace=trace,
                               trace_cores=trace_cores)
    out = np.zeros((B, T, D), np.float32)
    for c in range(N_CORES):
        out[c // 4] += np.asarray(res.results[c]["Z"]).astype(np.float32)
    return out, res


def kernel(x, Wq, Wk, Wv, Wo):
    try:
        out, _ = run(x, Wq, Wk, Wv, Wo)
    except Exception:
        # one retry for transient device errors (e.g. a wedged core from a
        # prior run)
        out, _ = run(x, Wq, Wk, Wv, Wo)
    return out


# revision 81
# speedup vs baseline: 1.0487x; 1.0216x over previous
"""Multi-head causal attention (B=2, T=2048, D=1024, H=16) on 8 TRN2
NeuronCores: data parallel over batch x tensor parallel over head groups
(4 heads per core). Each core computes its group's Q/K/V projections,
causal attention, and a partial output projection; the host sums the 4
partials per batch element.

v2: bf16 datapath (host-converted inputs), row-tiled S-matmul head pairs
(two 64-contraction matmuls run concurrently in the PE array), V^T
produced directly by swapping matmul operands (no PE transposes), fast
approximate reciprocal for the softmax denominators, paired broadcast
and divide.

Self-contained: builds the Bass/Tile kernel, runs it via
run_bass_kernel_spmd on cores 0-7, gathers on host.
"""
import numpy as np
import ml_dtypes

import concourse.bass as bass
import concourse.mybir as mybir
import concourse.tile as tile
from concourse.bass_utils import run_bass_kernel_spmd

P = 128
B, T, D = 2, 2048, 1024
H_LOCAL = 4          # heads per core
HD = 64              # head dim
F = H_LOCAL * HD     # 256 features per group
KO = D // P          # 8 contraction subtiles
NT = 512             # matmul moving width / PSUM bank
QJ = T // NT         # 4 q column tiles
KT = T // P          # 16 k row tiles
N_CORES = 8
LAG = 3              # S-matmul lookahead over P@V accumulation

f32 = mybir.dt.float32
f32r = mybir.dt.float32r
bf16 = mybir.dt.bfloat16

_uid = [0]


def _legalize_single_wait(nc):
    # This walrus build accepts only ONE sem wait per instruction; hoist
    # extra waits onto single-wait NoOps placed just before the instruction.
    for fn in nc.m.functions:
        for bb in fn.blocks:
            new_list = []
            changed = False
            for inst in bb.instructions:
                si = inst.sync_info
                if si is not None and len(si.on_wait) > 1:
                    waits = list(si.on_wait)
                    for w in waits[:-1]:
                        _uid[0] += 1
                        new_list.append(mybir.InstNoOp(
                            name=f"I-waitsplit-{_uid[0]}",
                            engine=inst.engine,
                            sync_info=mybir.SyncInfo(on_wait=[w], on_update=[]),
                        ))
                    inst.sync_info = mybir.SyncInfo(
                        on_wait=[waits[-1]], on_update=list(si.on_update))
                    changed = True
                new_list.append(inst)
            if changed:
                bb.instructions.clear()
                bb.instructions.extend(new_list)


def build_nc():
    nc = bass.Bass(trn_type="TRN2", target_bir_lowering=False, debug=False,
                   num_devices=N_CORES)
    xT = nc.dram_tensor("xT", [D, T], bf16, kind="ExternalInput").ap()
    WqT = nc.dram_tensor("WqT", [D, F], bf16, kind="ExternalInput").ap()
    WkT = nc.dram_tensor("WkT", [D, F], bf16, kind="ExternalInput").ap()
    WvT = nc.dram_tensor("WvT", [D, F], bf16, kind="ExternalInput").ap()
    WoT = nc.dram_tensor("WoT", [F, D], bf16, kind="ExternalInput").ap()
    TRI = nc.dram_tensor("TRI", [P, P], bf16, kind="ExternalInput").ap()
    Z = nc.dram_tensor("Z", [T, D], bf16, kind="ExternalOutput").ap()

    xTr = xT.rearrange("(ko p) t -> p ko t", p=P)
    w_r = {
        "q": WqT.rearrange("(ko p) f -> p ko f", p=P),
        "k": WkT.rearrange("(ko p) f -> p ko f", p=P),
        "v": WvT.rearrange("(ko p) f -> p ko f", p=P),
    }

    with tile.TileContext(nc) as tc:
        with (
            tc.tile_pool(name="cw", bufs=1) as cw,
            tc.tile_pool(name="sb1", bufs=1) as sb1,
            tc.tile_pool(name="tp", bufs=4) as tp,
            tc.tile_pool(name="psS", bufs=4, space="PSUM") as psS,
            tc.tile_pool(name="psO", bufs=2, space="PSUM") as psO,
            tc.tile_pool(name="psM", bufs=2, space="PSUM") as psM,
        ):
            # ---- persistent constants / staging ----
            w_sb = {}
            for name in ("q", "k", "v"):
                w_sb[name] = sb1.tile([P, KO, F], bf16, tag=f"w{name}",
                                      name=f"w{name}")
            xt = sb1.tile([P, KO, T], bf16, tag="xt", name="xt")
            # batched whole-tensor transfers (large DMAs spray across the
            # DMA engines; per-ko issues cost ~0.6us each on the queue);
            # weights dispatch on Sync, x on ACT (idle at startup) so the
            # descriptors go out in parallel
            nc.sync.dma_start(w_sb["q"][:], w_r["q"][:])
            nc.scalar.dma_start(xt[:, :, 0:NT], xTr[:, :, 0:NT])
            nc.sync.dma_start(w_sb["k"][:], w_r["k"][:])
            nc.scalar.dma_start(xt[:, :, NT:2 * NT], xTr[:, :, NT:2 * NT])
            nc.sync.dma_start(w_sb["v"][:], w_r["v"][:])
            for qj in range(2, QJ):
                nc.sync.dma_start(xt[:, :, qj * NT:(qj + 1) * NT],
                                  xTr[:, :, qj * NT:(qj + 1) * NT])

            wo = cw.tile([P, F // P, D], bf16, tag="wo", name="wo")
            nc.gpsimd.dma_start(wo[:], WoT.rearrange("(fo p) d -> p fo d", p=P))
            # causal mask replicated for the two heads of a pair
            tri2 = cw.tile([P, 2, P], bf16, tag="tri2", name="tri2")
            nc.gpsimd.dma_start(tri2[:, 0], TRI)
            nc.gpsimd.dma_start(tri2[:, 1], TRI)
            ones33 = cw.tile([1, HD], mybir.dt.float16, tag="ones33",
                             name="ones33")
            nc.gpsimd.memset(ones33[:], 1.0)

            # Q/K^T for head pair p: rows 0:64 = head 2p, rows 64:128 = head
            # 2p+1 (the projection psum layout, verbatim).
            qt = cw.tile([P, 2, T], bf16, tag="qt", name="qt")
            kt2 = cw.tile([P, 2, T], bf16, tag="kt2", name="kt2")

            # V^T with a ones column per head: [k-token, kt, head, 0:64]=V^T,
            # [..., 64]=1 (gives softmax denominators for free in P@V)
            vaug = cw.tile([P, KT, H_LOCAL, HD + 1], bf16, tag="vaug",
                           name="vaug")
            nc.gpsimd.memset(vaug[:, :, :, HD:HD + 1], 1.0)

            ot = cw.tile([P, F // P, T], bf16, tag="ot", name="ot")

            def phase1_chunks(qj):
                # emission chunks (each ~8 PE matmuls) to splice into the
                # attention stream so the PE never drains
                sl = slice(qj * NT, (qj + 1) * NT)
                chunks = []

                def proj(name, fs):
                    def emit():
                        ps = psM.tile([P, NT], f32, tag="m",
                                      name=f"ps_{name}{fs}_{qj}")
                        for ko in range(KO):
                            nc.tensor.matmul(
                                ps[:], w_sb[name][:, ko, fs * P:(fs + 1) * P],
                                xt[:, ko, sl],
                                start=(ko == 0), stop=(ko == KO - 1))
                        dst = qt if name == "q" else kt2
                        nc.vector.tensor_copy(dst[:, fs, sl], ps[:])
                    return emit

                def vproj(kt):
                    # V^T directly: stationary = x tile, moving = Wv.
                    # out[tok, f] = sum_d x[kt*128+tok, d] * Wv[f, d]
                    def emit():
                        pv = psM.tile([P, F], f32, tag="m", name=f"pv{kt}")
                        for ko in range(KO):
                            nc.tensor.matmul(
                                pv[:], xt[:, ko, kt * P:(kt + 1) * P],
                                w_sb["v"][:, ko, :],
                                start=(ko == 0), stop=(ko == KO - 1))
                        nc.vector.tensor_copy(
                            vaug[:, kt, :, 0:HD],
                            pv.rearrange("p (h d) -> p h d", h=H_LOCAL))
                    return emit

                for name in ("q", "k"):
                    for fs in range(F // P):
                        chunks.append(proj(name, fs))
                for kt in range(4 * qj, 4 * qj + 4):
                    chunks.append(vproj(kt))
                return chunks

            def phase23_pair(p, qj, pending, splice):
                # two heads (2p, 2p+1) processed together: their S matmuls
                # are 64-contraction row-tiles (partitions 0:64 / 64:128)
                # that run concurrently in the PE array.
                po = [psO.tile([HD + 1, NT], f32, tag="o",
                               name=f"po{p}_{qj}_{e}") for e in (0, 1)]
                n_ki = 4 * qj + 4
                pts = {}

                def s_step(ki):
                    col0 = 0 if ki < 4 * qj else (ki - 4 * qj) * P
                    N = NT - col0
                    kb = slice(ki * P, (ki + 1) * P)
                    qs = slice(qj * NT + col0, (qj + 1) * NT)
                    # the pair's S matmuls write one 2-bank PSUM tile: both
                    # banks recycle together, so the two row-tiled matmuls
                    # become ready together and run concurrently; ONE merged
                    # exp and ONE merged mask-mul cover both heads
                    ps = psS.tile([P, 2, NT], f32, tag="s", bufs=2,
                                  name=f"pss{p}_{qj}_{ki}")
                    for e in (0, 1):
                        rows = slice(HD * e, HD * e + HD)
                        nc.tensor.matmul(
                            ps[:, e, 0:N], kt2[rows, p, kb], qt[rows, p, qs],
                            start=True, stop=True)
                    pt = tp.tile([P, 2, NT], bf16, tag="pt", bufs=4,
                                 name=f"pt{p}_{qj}_{ki}")
                    nc.scalar.activation(pt[:, :, 0:N], ps[:, :, 0:N],
                                         mybir.ActivationFunctionType.Exp,
                                         scale=0.125)
                    if ki >= 4 * qj:
                        nc.vector.tensor_mul(pt[:, :, 0:P], pt[:, :, 0:P],
                                             tri2[:])
                    pts[ki] = (pt, col0, N)

                def o_step(ki):
                    pt, col0, N = pts.pop(ki)
                    for e in (0, 1):
                        nc.tensor.matmul(
                            po[e][:, col0:NT], vaug[:, ki, 2 * p + e, :],
                            pt[:, e, 0:N],
                            start=(ki == 0), stop=(ki == n_ki - 1))

                # splice points: external chunks between ki steps. The
                # first LAG iterations have no o_steps (PE would sit ~50%
                # idle there and HAM can re-throttle), so they get a double
                # share of the chunks.
                nst = n_ki + LAG
                wts = [2 if ki < LAG else 1 for ki in range(nst)]
                tot = sum(wts)
                cum = [0]
                for w in wts:
                    cum.append(cum[-1] + w)
                nsp = len(splice)

                for ki in range(nst):
                    if ki == LAG and pending is not None:
                        # previous pair's division, emitted here so its PE
                        # broadcast never heads the PE stream while waiting
                        # on the ACT reciprocal chain (and before o_step(0)
                        # overwrites the rotated po buffers)
                        pending()
                        pending = None
                    # o_step before s_step: the PV depends on an OLDER exp
                    # (ki-LAG) than the S-pair does (ki-2), so it must not
                    # sit behind the S-pair in the in-order PE queue
                    if ki >= LAG:
                        o_step(ki - LAG)
                    if ki < n_ki:
                        s_step(ki)
                    for c in splice[(nsp * cum[ki]) // tot:
                                    (nsp * cum[ki + 1]) // tot]:
                        c()
                if pending is not None:
                    pending()

                def division():
                    # numerators of both heads stacked [128, NT]; raw
                    # denominator rows broadcast across partitions by two
                    # col-tiled PE matmuls; one paired DVE divide into OT.
                    sp = tp.tile([P, NT], f32, tag="so", bufs=2,
                                 name=f"sp{p}_{qj}")
                    rrs = []
                    for e in (0, 1):
                        nc.vector.tensor_copy(sp[HD * e:HD * e + HD, :],
                                              po[e][0:HD, :])
                        # 1/d = exp(-ln d) on ACT (DVE reciprocal is serial
                        # per-lane; custom DVE ops unsupported in this build)
                        ll = tp.tile([1, NT], f32, tag=f"ll{e}", bufs=2,
                                     name=f"ll{p}_{qj}_{e}")
                        nc.scalar.activation(ll[:], po[e][HD:HD + 1, :],
                                             mybir.ActivationFunctionType.Ln)
                        rr = tp.tile([1, NT], mybir.dt.float16, tag=f"rr{e}",
                                     bufs=2, name=f"rr{p}_{qj}_{e}")
                        nc.scalar.activation(rr[:], ll[:],
                                             mybir.ActivationFunctionType.Exp,
                                             scale=-1.0)
                        rrs.append(rr)
                    pb = psM.tile([P, NT], f32, tag="m", name=f"pb{p}_{qj}")
                    for e in (0, 1):
                        nc.tensor.matmul(pb[HD * e:HD * e + HD, :],
                                         ones33[0:1, :], rrs[e][:],
                                         start=True, stop=True)
                    nc.vector.tensor_mul(ot[:, p, qj * NT:(qj + 1) * NT],
                                         sp[:], pb[:])
                return division

            def phase4(qt_i):
                for dt in range(D // NT):
                    pz = psM.tile([P, NT], f32, tag="m", name=f"pz{qt_i}_{dt}")
                    for fs in range(F // P):
                        nc.tensor.matmul(
                            pz[:], ot[:, fs, qt_i * P:(qt_i + 1) * P],
                            wo[:, fs, dt * NT:(dt + 1) * NT],
                            start=(fs == 0), stop=(fs == F // P - 1))
                    zs = tp.tile([P, NT], bf16, tag="z", bufs=2,
                                 name=f"zs{qt_i}_{dt}")
                    nc.vector.tensor_copy(zs[:], pz[:])
                    nc.sync.dma_start(
                        Z[qt_i * P:(qt_i + 1) * P, dt * NT:(dt + 1) * NT],
                        zs[:])

            # chunk order within a splice: fs0 projections first (feed the
            # NEXT qj's pair0), early V tiles before the o_steps that read
            # them, fs1 projections in the pair1 half.
            def ordered(ch):
                return [ch[0], ch[2], ch[4], ch[5], ch[1], ch[3], ch[6], ch[7]]

            pending = None
            ch0 = phase1_chunks(0)
            for c in (ch0[0], ch0[2], ch0[4]):   # q fs0, k fs0, V kt0
                c()
            rest0 = [ch0[5], ch0[6], ch0[7], ch0[1], ch0[3]]
            p4_backlog = []
            for qj in range(QJ):
                splice = ordered(phase1_chunks(qj + 1)) if qj + 1 < QJ else []
                if qj == 2:
                    splice += p4_backlog[:4]       # phase4 of qj 0
                    p4_backlog = p4_backlog[4:]
                elif qj == 3:
                    splice += p4_backlog[:8]       # phase4 of qj 1 and 2 —
                    p4_backlog = p4_backlog[8:]    # keep the tail short and
                                                   # dense so HAM stays warm
                if qj == 0:
                    # minimal warmup happened above; attention starts ASAP
                    pending = phase23_pair(0, 0, pending, rest0)
                    pending = phase23_pair(1, 0, pending, splice)
                else:
                    for p in range(2):
                        k0 = (len(splice) * p) // 2
                        k1 = (len(splice) * (p + 1)) // 2
                        pending = phase23_pair(p, qj, pending, splice[k0:k1])
                p4_backlog += [(lambda qt_i=qt_i: phase4(qt_i))
                               for qt_i in range(4 * qj, 4 * qj + 4)]
            if pending is not None:
                pending()
            for c in p4_backlog:
                c()

    _legalize_single_wait(nc)
    return nc


_TRI = None


def _make_in_maps(x, Wq, Wk, Wv, Wo):
    global _TRI
    bf = ml_dtypes.bfloat16
    if _TRI is None:
        # allowed[k_row, q_col] = q >= k  (upper-triangular incl. diagonal)
        _TRI = (np.arange(P)[None, :] >= np.arange(P)[:, None]).astype(bf)
    in_maps = []
    for c in range(N_CORES):
        b, g = divmod(c, 4)
        sl = slice(g * F, (g + 1) * F)
        in_maps.append({
            "xT": np.ascontiguousarray(np.asarray(x)[b].T).astype(bf),
            "WqT": np.ascontiguousarray(np.asarray(Wq)[sl, :].T).astype(bf),
            "WkT": np.ascontiguousarray(np.asarray(Wk)[sl, :].T).astype(bf),
            "WvT": np.ascontiguousarray(np.asarray(Wv)[sl, :].T).astype(bf),
            "WoT": np.ascontiguousarray(np.asarray(Wo)[:, sl].T).astype(bf),
            "TRI": _TRI,
        })
    return in_maps


def run(x, Wq, Wk, Wv, Wo, trace=False, trace_cores=None):
    nc = build_nc()
    in_maps = _make_in_maps(x, Wq, Wk, Wv, Wo)
    res = run_bass_kernel_spmd(nc, in_maps, list(range(N_CORES)), trace=trace,
                               trace_cores=trace_cores)
    out = np.zeros((B, T, D), np.float32)
    for c in range(N_CORES):
        out[c // 4] += np.asarray(res.results[c]["Z"]).astype(np.float32)
    return out, res


def kernel(x, Wq, Wk, Wv, Wo):
    try:
        out, _ = run(x, Wq, Wk, Wv, Wo)
    except Exception:
        # one retry for transient device errors (e.g. a wedged core from a
        # prior run)
        out, _ = run(x, Wq, Wk, Wv, Wo)
    return out
